# revision 12
# baseline (speedup 1.0000x reference)
"""Trainium2 Bass kernel for nn_CBFHalfspace.

The reference module computes, per 7-vector state x:
    h     = b - A @ x          (4,)   with A rows [[-1,0,...],[1,0,...],[0,-1,...],[0,1,...]], b = ones
    Lfh   = grad(sum h) @ f    scalar -- grad is -A.sum(0) == 0, so Lfh == 0
    Lf2h  = 0, LgLfh = (0, 0)  (second-order grads of an affine map)
so the full output row is [1+x0, 1-x0, 1+x1, 1-x1, 0, 0, 0, 0] and f/g are
unused. That makes this a pure memory-streaming problem: read x (B,7),
write out (B,8), roofline-bound by HBM bandwidth.

Layout: each core handles B/8 contiguous rows. A tile loads 128*T rows as
[128 partitions x 7T] (each partition holds T whole rows, contiguous in
DRAM), computes the interleaved [128 x 8T] output with two strided DVE
tensor_scalar ops (the +/- affine columns; the zero columns are memset
once per buffer before the loop), and streams it back out. All DMAs are
fully contiguous per partition and multi-MB.

Raw Bass (no Tile): explicit semaphores keep every instruction within the
ISA's tiny per-instruction sync budget (standalone wait_ge instructions),
input DMAs issue from the ACT HWDGE FIFO and output DMAs from the SP HWDGE
FIFO so they pipeline independently against the DVE compute.
"""

import numpy as np

import concourse.bass as bass
import concourse.mybir as mybir
from concourse.bass_utils import run_bass_kernel_spmd

N_CORES = 8
B = 4_194_304
N_PER_CORE = B // N_CORES  # 524288
P = 128
T = 1024                   # rows per partition per tile
NBUF = 2
F32 = mybir.dt.float32
ADD = mybir.AluOpType.add
MULT = mybir.AluOpType.mult


def build_kernel(n_rows: int = N_PER_CORE, t: int = T, nbuf: int = NBUF,
                 repeat: int = 1) -> bass.Bass:
    """repeat>1 re-streams the same tiles that many times (idempotent
    writes) — used only for benchmarking, to amortize dispatch overhead."""
    rows_per_tile = P * t
    assert n_rows % rows_per_tile == 0
    ntiles = n_rows // rows_per_tile
    assert ntiles >= nbuf
    niter = ntiles * repeat

    nc = bass.Bass()
    x = nc.dram_tensor("x", [n_rows, 7], F32, kind="ExternalInput")
    out = nc.dram_tensor("out", [n_rows, 8], F32, kind="ExternalOutput")

    x_t = x[:].flatten().rearrange("(n p m) -> n p m", p=P, m=7 * t)
    out_t = out[:].flatten().rearrange("(n p m) -> n p m", p=P, m=8 * t)

    from contextlib import ExitStack

    with ExitStack() as ctx:
        xbuf = ctx.enter_context(nc.sbuf_tensor([P, 7 * t * nbuf], F32))
        obuf = ctx.enter_context(nc.sbuf_tensor([P, 8 * t * nbuf], F32))
        # one in/out semaphore per buffer slot: at most one DMA per slot is
        # in flight, so sem values are unambiguous (a single shared sem
        # would interleave the per-SDMA-engine +1s of concurrent DMAs)
        s_in = [ctx.enter_context(nc.semaphore(f"s_in{b}")) for b in range(nbuf)]
        s_out = [ctx.enter_context(nc.semaphore(f"s_out{b}")) for b in range(nbuf)]
        s_cmp = ctx.enter_context(nc.semaphore("s_cmp"))
        block = ctx.enter_context(nc.Block())

        xts = [xbuf[:, b * 7 * t:(b + 1) * 7 * t] for b in range(nbuf)]
        ots = [obuf[:, b * 8 * t:(b + 1) * 8 * t] for b in range(nbuf)]

        @block.scalar
        def _(act):
            # input DMAs (HWDGE via ACT sequencer)
            for i in range(niter):
                b = i % nbuf
                if i >= nbuf:
                    # xt[b] may be overwritten once compute of i-nbuf retired
                    act.wait_ge(s_cmp, (i - nbuf) + 1)
                act.dma_start(out=xts[b], in_=x_t[i % ntiles]).then_inc(s_in[b], 16)

        @block.sync
        def _(sp):
            # output DMAs (HWDGE via SP sequencer)
            for i in range(niter):
                b = i % nbuf
                sp.wait_ge(s_cmp, i + 1)
                sp.dma_start(out=out_t[i % ntiles], in_=ots[b]).then_inc(s_out[b], 16)
            # make sure the final stores have landed before the program ends
            for b in range(nbuf):
                uses = len(range(b, niter, nbuf))
                sp.wait_ge(s_out[b], 16 * uses)

        @block.vector
        def _(dve):
            # columns 4..7 of every output row are identically zero; write
            # them once per buffer. The loop only touches columns 0..3.
            for b in range(nbuf):
                o3 = ots[b].rearrange("p (t j) -> p t j", j=8)
                dve.memset(o3[:, :, 4:8], 0.0)
            for i in range(niter):
                b = i % nbuf
                rnd = i // nbuf
                dve.wait_ge(s_in[b], 16 * (rnd + 1))
                if i >= nbuf:
                    # ot[b] may be rewritten once its previous store is done
                    dve.wait_ge(s_out[b], 16 * rnd)
                x3 = xts[b].rearrange("p (t k) -> p t k", k=7)
                o3 = ots[b].rearrange("p (t j) -> p t j", j=8)
                # out[:, 0], out[:, 2] = 1 + x0, 1 + x1
                dve.tensor_scalar(o3[:, :, 0:4:2], x3[:, :, 0:2], 1.0, None, ADD)
                # out[:, 1], out[:, 3] = 1 - x0, 1 - x1
                dve.tensor_scalar(
                    o3[:, :, 1:4:2], x3[:, :, 0:2], -1.0, 1.0, MULT, ADD
                ).then_inc(s_cmp, 1)

    return nc


_NC_CACHE: dict = {}


def _get_nc() -> bass.Bass:
    key = (N_PER_CORE, T, NBUF)
    if key not in _NC_CACHE:
        _NC_CACHE[key] = build_kernel()
    return _NC_CACHE[key]


def run(x: np.ndarray, trace: bool = False):
    """Run on 8 cores; returns (out (B,8) float32, BassKernelResults)."""
    x = np.ascontiguousarray(np.asarray(x, dtype=np.float32))
    assert x.shape == (B, 7)
    shards = np.split(x, N_CORES, axis=0)
    in_maps = [{"x": s} for s in shards]
    res = run_bass_kernel_spmd(
        _get_nc(), in_maps, list(range(N_CORES)), trace=trace
    )
    out = np.concatenate([r["out"] for r in res.results], axis=0)
    return out, res


def kernel(x: np.ndarray, f: np.ndarray = None, g: np.ndarray = None, **_) -> np.ndarray:
    # f and g do not influence the output (all Lie-derivative terms are
    # exactly zero for this affine barrier); accepted for API compatibility.
    out, _res = run(x)
    return out


# revision 13
# speedup vs baseline: 1.0054x; 1.0054x over previous
"""Trainium2 Bass kernel for nn_CBFHalfspace.

The reference module computes, per 7-vector state x:
    h     = b - A @ x          (4,)   with A rows [[-1,0,...],[1,0,...],[0,-1,...],[0,1,...]], b = ones
    Lfh   = grad(sum h) @ f    scalar -- grad is -A.sum(0) == 0, so Lfh == 0
    Lf2h  = 0, LgLfh = (0, 0)  (second-order grads of an affine map)
so the full output row is [1+x0, 1-x0, 1+x1, 1-x1, 0, 0, 0, 0] and f/g are
unused. That makes this a pure memory-streaming problem: read x (B,7),
write out (B,8), roofline-bound by HBM bandwidth.

Layout: each core handles B/8 contiguous rows. A tile loads 128*T rows as
[128 partitions x 7T] (each partition holds T whole rows, contiguous in
DRAM), computes the interleaved [128 x 8T] output with two strided DVE
tensor_scalar ops (the +/- affine columns; the zero columns are memset
once per buffer before the loop), and streams it back out. All DMAs are
fully contiguous per partition and multi-MB.

Raw Bass (no Tile): explicit semaphores keep every instruction within the
ISA's tiny per-instruction sync budget (standalone wait_ge instructions),
input DMAs issue from the ACT HWDGE FIFO and output DMAs from the SP HWDGE
FIFO so they pipeline independently against the DVE compute.
"""

import numpy as np

import concourse.bass as bass
import concourse.mybir as mybir
from concourse.bass_utils import run_bass_kernel_spmd

N_CORES = 8
B = 4_194_304
N_PER_CORE = B // N_CORES  # 524288
P = 128
T = 1024                   # rows per partition per tile
NBUF = 2
F32 = mybir.dt.float32
ADD = mybir.AluOpType.add
MULT = mybir.AluOpType.mult


def build_kernel(n_rows: int = N_PER_CORE, t: int = T, nbuf: int = NBUF,
                 repeat: int = 1, tile_sizes: list | None = None,
                 serialize_reps: bool = False) -> bass.Bass:
    """repeat>1 re-streams the same tiles that many times (idempotent
    writes) — used only for benchmarking, to amortize dispatch overhead.
    serialize_reps makes each rep wait for the previous rep's final store,
    so a repeat-diff measures isolated single-execution time (fill+tail
    included) instead of steady-state chaining.
    tile_sizes: optional explicit per-tile row counts (units of P rows),
    summing to n_rows//P; slots are sized for max(tile_sizes)."""
    if tile_sizes is None:
        rows_per_tile = P * t
        assert n_rows % rows_per_tile == 0
        tile_sizes = [t] * (n_rows // rows_per_tile)
    assert sum(tile_sizes) * P == n_rows
    ntiles = len(tile_sizes)
    assert ntiles >= nbuf
    tmax = max(tile_sizes)
    niter = ntiles * repeat

    nc = bass.Bass()
    x = nc.dram_tensor("x", [n_rows, 7], F32, kind="ExternalInput")
    out = nc.dram_tensor("out", [n_rows, 8], F32, kind="ExternalOutput")

    x_flat = x[:].flatten()
    out_flat = out[:].flatten()
    x_t, out_t = [], []
    off = 0  # in units of P rows
    for tt in tile_sizes:
        x_t.append(
            x_flat[off * P * 7:(off + tt) * P * 7].rearrange("(p m) -> p m", p=P)
        )
        out_t.append(
            out_flat[off * P * 8:(off + tt) * P * 8].rearrange("(p m) -> p m", p=P)
        )
        off += tt

    from contextlib import ExitStack

    with ExitStack() as ctx:
        xbuf = ctx.enter_context(nc.sbuf_tensor([P, 7 * tmax * nbuf], F32))
        obuf = ctx.enter_context(nc.sbuf_tensor([P, 8 * tmax * nbuf], F32))
        # one in/out semaphore per buffer slot: at most one DMA per slot is
        # in flight, so sem values are unambiguous (a single shared sem
        # would interleave the per-SDMA-engine +1s of concurrent DMAs)
        s_in = [ctx.enter_context(nc.semaphore(f"s_in{b}")) for b in range(nbuf)]
        s_out = [ctx.enter_context(nc.semaphore(f"s_out{b}")) for b in range(nbuf)]
        s_cmp = ctx.enter_context(nc.semaphore("s_cmp"))
        block = ctx.enter_context(nc.Block())

        xts = [xbuf[:, b * 7 * tmax:(b + 1) * 7 * tmax] for b in range(nbuf)]
        ots = [obuf[:, b * 8 * tmax:(b + 1) * 8 * tmax] for b in range(nbuf)]

        def tsz(i):
            return tile_sizes[i % ntiles]

        @block.scalar
        def _(act):
            # input DMAs (HWDGE via ACT sequencer)
            for i in range(niter):
                b = i % nbuf
                if serialize_reps and i % ntiles == 0 and i > 0:
                    # isolate executions: wait for the previous rep's stores
                    for bb in range(nbuf):
                        done = len(range(bb, i, nbuf))
                        act.wait_ge(s_out[bb], 16 * done)
                if i >= nbuf:
                    # xt[b] may be overwritten once compute of i-nbuf retired
                    act.wait_ge(s_cmp, (i - nbuf) + 1)
                tt = tsz(i)
                act.dma_start(out=xts[b][:, :7 * tt],
                              in_=x_t[i % ntiles]).then_inc(s_in[b], 16)

        @block.sync
        def _(sp):
            # output DMAs (HWDGE via SP sequencer)
            for i in range(niter):
                b = i % nbuf
                sp.wait_ge(s_cmp, i + 1)
                sp.dma_start(out=out_t[i % ntiles],
                             in_=ots[b][:, :8 * tsz(i)]).then_inc(s_out[b], 16)
            # make sure the final stores have landed before the program ends
            for b in range(nbuf):
                uses = len(range(b, niter, nbuf))
                sp.wait_ge(s_out[b], 16 * uses)

        @block.vector
        def _(dve):
            # columns 4..7 of every output row are identically zero; write
            # them once per buffer. The loop only touches columns 0..3.
            for b in range(nbuf):
                o3 = ots[b].rearrange("p (t j) -> p t j", j=8)
                dve.memset(o3[:, :, 4:8], 0.0)
            for i in range(niter):
                b = i % nbuf
                rnd = i // nbuf
                dve.wait_ge(s_in[b], 16 * (rnd + 1))
                if i >= nbuf:
                    # ot[b] may be rewritten once its previous store is done
                    dve.wait_ge(s_out[b], 16 * rnd)
                tt = tsz(i)
                x3 = xts[b][:, :7 * tt].rearrange("p (t k) -> p t k", k=7)
                o3 = ots[b][:, :8 * tt].rearrange("p (t j) -> p t j", j=8)
                # out[:, 0], out[:, 2] = 1 + x0, 1 + x1
                dve.tensor_scalar(o3[:, :, 0:4:2], x3[:, :, 0:2], 1.0, None, ADD)
                # out[:, 1], out[:, 3] = 1 - x0, 1 - x1
                dve.tensor_scalar(
                    o3[:, :, 1:4:2], x3[:, :, 0:2], -1.0, 1.0, MULT, ADD
                ).then_inc(s_cmp, 1)

    return nc


_NC_CACHE: dict = {}


def _get_nc() -> bass.Bass:
    key = (N_PER_CORE, T, NBUF)
    if key not in _NC_CACHE:
        _NC_CACHE[key] = build_kernel()
    return _NC_CACHE[key]


def run(x: np.ndarray, trace: bool = False):
    """Run on 8 cores; returns (out (B,8) float32, BassKernelResults)."""
    x = np.ascontiguousarray(np.asarray(x, dtype=np.float32))
    assert x.shape == (B, 7)
    shards = np.split(x, N_CORES, axis=0)
    in_maps = [{"x": s} for s in shards]
    res = run_bass_kernel_spmd(
        _get_nc(), in_maps, list(range(N_CORES)), trace=trace
    )
    out = np.concatenate([r["out"] for r in res.results], axis=0)
    return out, res


def kernel(x: np.ndarray, f: np.ndarray = None, g: np.ndarray = None, **_) -> np.ndarray:
    # f and g do not influence the output (all Lie-derivative terms are
    # exactly zero for this affine barrier); accepted for API compatibility.
    out, _res = run(x)
    return out
